# revision 13
# baseline (speedup 1.0000x reference)
"""Trainium2 Bass kernel for GAT-style attention score computation.

Math (see reference):
    s_src = X @ a[:F];  s_dst = X @ a[F:]
    e[i, j] = leaky_relu(s_src[i] + s_dst[j], alpha=0.2)

Sharding over 8 NeuronCores: row-shard the N x N output (1024 rows/core).
No collective: each core receives a replicated fp16 copy of X^T and
computes the full s_dst row-vector itself with full-rate fp16 matmuls
whose ones-like stationary operand broadcasts s_dst across all 128
partitions directly (d_bcast).  The local s_src slice comes from an
f32 row shard via fused DVE multiply-reduce.

Per-core dataflow (half = 4096 output columns):
  - a_cols [128,4] f32 via a strided DMA of the attention vector; the
    stationary lhsT tiles are ones*a_dst chunks (tensor_scalar).
  - per half: 8x2 fp16 matmuls (K=256 in two 128-passes) -> PSUM
    [128,512] holding s_dst replicated across partitions -> DVE copy
    casts to an fp16 d_bcast [128,4096] tile.
  - main loop per half: 8 sub-rows t; row r=t*128+p of the local block.
      ACT rows: out = Prelu(d_bcast + s_src[:,t]) in one activation.
      DVE rows: u = d+s; v = 0.2*d+0.2*s; out = max(u,v).
    Output tile [128,4096] fp16 -> DMA (8 KiB/partition lines, 1 MiB).

Output is written in fp16 (harness gate is 2e-2 rel err; fp16 keeps it
~1e-4) and upcast to f32 on the host, halving the dominant HBM write
traffic.  Roofline: ~22 MB DMA/core at ~358 GB/s -> ~61 us floor.
"""

import numpy as np

N = 8192
F = 256
NCORES = 8
ROWS = N // NCORES          # 1024 rows per core
P = 128                     # partitions
C = ROWS // P               # 8 sub-rows per partition
ALPHA = 0.2
HALF = N // 2               # 4096 columns per half
QCH = 512                   # matmul free-dim chunk (one PSUM bank)
NACT = 5                    # sub-rows on the scalar engine (rest on DVE)

_CACHE = {}


def _build():
    import concourse.bacc as bacc
    import concourse.bass as bass
    import concourse.tile as tile
    from concourse import mybir

    fp32 = mybir.dt.float32
    fp16 = mybir.dt.float16

    nc = bacc.Bacc(
        "TRN2",
        target_bir_lowering=False,
        debug=False,
        num_devices=NCORES,
    )

    xt_dram = nc.dram_tensor("xt", [F, N], fp16, kind="ExternalInput")
    xloc_dram = nc.dram_tensor("xloc", [P, C * F], fp32, kind="ExternalInput")
    av_dram = nc.dram_tensor("av", [2 * F, 1], fp32, kind="ExternalInput")
    out_dram = nc.dram_tensor("out", [ROWS, N], fp16, kind="ExternalOutput")

    with tile.TileContext(nc) as tc:
        with (
            tc.tile_pool(name="const", bufs=1) as const_pool,
            tc.tile_pool(name="xt", bufs=2) as xt_pool,
            tc.tile_pool(name="work", bufs=2) as work_pool,
            tc.tile_pool(name="dbc", bufs=2) as dbc_pool,
            tc.tile_pool(name="uv", bufs=4) as uv_pool,
            tc.tile_pool(name="outp", bufs=4) as out_pool,
            tc.tile_pool(name="psum", bufs=4, space=bass.MemorySpace.PSUM) as psum_pool,
            tc.tile_pool(name="psA", bufs=2, space=bass.MemorySpace.PSUM) as psA_pool,
        ):
            # ---- X^T halves first: they gate d_bcast -> the main loop ----
            xtA = [xt_pool.tile([P, HALF], fp16, tag="xtA", name=f"xtA{h}") for h in range(2)]
            xtB = [xt_pool.tile([P, HALF], fp16, tag="xtB", name=f"xtB{h}") for h in range(2)]
            xt_ap = xt_dram.ap()
            for h in range(2):
                nc.sync.dma_start(xtA[h][:], xt_ap[0:P, h * HALF:(h + 1) * HALF])
                nc.sync.dma_start(xtB[h][:], xt_ap[P:2 * P, h * HALF:(h + 1) * HALF])

            # ---- tiny input loads on the scalar-engine HWDGE ring, off the
            # xt/output queue; a_cols first (it gates lhsT -> the matmuls) ----
            # a_cols[f, a] = av[a*128 + f]; cols: 0,1 = a_src, 2,3 = a_dst
            a_cols = const_pool.tile([P, 4], fp32)
            nc.scalar.dma_start(
                a_cols[:], av_dram.ap().rearrange("(a f) one -> f (a one)", f=P)
            )
            # flat row copy of av for the a_src partition-broadcast matmul
            av_row = const_pool.tile([1, 2 * F], fp32)
            nc.scalar.dma_start(av_row[:], av_dram.ap().rearrange("f one -> one f"))

            xloc_sb = const_pool.tile([P, C * F], fp32)
            nc.scalar.dma_start(xloc_sb[:], xloc_dram.ap())

            # ---- stationary lhsT tiles: ones * a_dst chunk ----
            ones16 = const_pool.tile([P, P], fp16)
            nc.vector.memset(ones16[:], 1.0)
            lhsT = []
            for kb in range(2):
                t_ = const_pool.tile([P, P], fp16, tag=f"lhsT{kb}", name=f"lhsT{kb}")
                nc.vector.tensor_scalar(
                    t_[:], ones16[:], a_cols[:, 2 + kb:3 + kb], None,
                    op0=mybir.AluOpType.mult,
                )
                lhsT.append(t_)

            # ---- a_src broadcast across partitions (for the s_src matvec) ----
            ones1 = const_pool.tile([1, P], fp32)
            nc.vector.memset(ones1[:], 1.0)
            asrc_ps = psum_pool.tile([P, F], fp32, tag="asrc")
            nc.tensor.matmul(
                asrc_ps[:], ones1[:], av_row[0:1, 0:F], start=True, stop=True
            )
            ab_src = const_pool.tile([P, F], fp32)
            nc.vector.tensor_copy(ab_src[:], asrc_ps[:])

            # ---- s_src: GpSimd multiplies (free-dim reduce must run on the
            # vector engine; those are emitted after the first d_bcast
            # copies so they don't stall the DVE FIFO) ----
            # (tensor_tensor_reduce faults real HW; keep the two-op form)
            s_src = const_pool.tile([P, C], fp32)
            mv_scratch = const_pool.tile([P, C * F], fp32)
            for c in range(C):
                nc.gpsimd.tensor_tensor(
                    mv_scratch[:, c * F:(c + 1) * F],
                    xloc_sb[:, c * F:(c + 1) * F], ab_src[:],
                    op=mybir.AluOpType.mult,
                )

            def emit_ssrc_reduces():
                for c in range(C):
                    nc.vector.tensor_reduce(
                        s_src[:, c:c + 1], mv_scratch[:, c * F:(c + 1) * F],
                        axis=mybir.AxisListType.X, op=mybir.AluOpType.add,
                    )

            s_srcA = const_pool.tile([P, C], fp32)

            out_view = out_dram.ap().rearrange("(c p) n -> p c n", c=C)

            for h in range(2):
                # ---- d_bcast[h]: s_dst replicated across partitions, fp16 ----
                dbc = dbc_pool.tile([P, HALF], fp16, tag="dbc")
                for q in range(HALF // QCH):
                    ps = psA_pool.tile([P, QCH], fp32, tag="dps")
                    sl = slice(q * QCH, (q + 1) * QCH)
                    nc.tensor.matmul(
                        ps[:], lhsT[0][:], xtA[h][:, sl], start=True, stop=False
                    )
                    nc.tensor.matmul(
                        ps[:], lhsT[1][:], xtB[h][:, sl], start=False, stop=True
                    )
                    nc.vector.tensor_copy(dbc[:, sl], ps[:])

                if h == 0:
                    emit_ssrc_reduces()
                    nc.gpsimd.tensor_scalar(
                        s_srcA[:], s_src[:], ALPHA, None,
                        op0=mybir.AluOpType.mult,
                    )

                # ---- main loop: 8 sub-rows over this half ----
                csl = slice(h * HALF, (h + 1) * HALF)
                for t in range(C):
                    o = out_pool.tile([P, HALF], fp16)
                    if t < NACT:
                        nc.scalar.activation(
                            o[:],
                            dbc[:],
                            mybir.ActivationFunctionType.Prelu,
                            bias=s_src[:, t:t + 1],
                            scale=1.0,
                            alpha=ALPHA,
                        )
                    else:
                        u = uv_pool.tile([P, HALF], fp16, tag="u")
                        v = uv_pool.tile([P, HALF], fp16, tag="v")
                        nc.vector.tensor_scalar(
                            u[:], dbc[:], s_src[:, t:t + 1], None,
                            op0=mybir.AluOpType.add,
                        )
                        nc.vector.tensor_scalar(
                            v[:], dbc[:], ALPHA, s_srcA[:, t:t + 1],
                            op0=mybir.AluOpType.mult, op1=mybir.AluOpType.add,
                        )
                        nc.vector.tensor_tensor(
                            o[:], u[:], v[:], op=mybir.AluOpType.max
                        )
                    nc.sync.dma_start(out_view[:, t, csl], o[:])

    nc.compile()
    return nc


def _get_nc():
    if "nc" not in _CACHE:
        _CACHE["nc"] = _build()
    return _CACHE["nc"]


def build_in_maps(feature_matrix: np.ndarray, attention_vector: np.ndarray):
    feature_matrix = np.ascontiguousarray(feature_matrix, dtype=np.float32)
    attention_vector = np.ascontiguousarray(attention_vector, dtype=np.float32)
    xt = np.ascontiguousarray(feature_matrix.T.astype(np.float16))
    in_maps = []
    for c in range(NCORES):
        shard = feature_matrix[c * ROWS:(c + 1) * ROWS]
        # partition p, sub-row chunk c2 holds local row c2*128 + p
        xloc = np.ascontiguousarray(
            shard.reshape(C, P, F).transpose(1, 0, 2).reshape(P, C * F)
        )
        in_maps.append({"xt": xt, "xloc": xloc, "av": attention_vector})
    return in_maps


def kernel(feature_matrix: np.ndarray, attention_vector: np.ndarray) -> np.ndarray:
    from concourse.bass_utils import run_bass_kernel_spmd

    nc = _get_nc()
    in_maps = build_in_maps(feature_matrix, attention_vector)
    res = run_bass_kernel_spmd(nc, in_maps, core_ids=list(range(NCORES)))
    out = np.concatenate(
        [res.results[c]["out"] for c in range(NCORES)], axis=0
    )
    return out.astype(np.float32)


# revision 16
# speedup vs baseline: 1.1222x; 1.1222x over previous
"""Trainium2 Bass kernel for GAT-style attention score computation.

Math (see reference):
    s_src = X @ a[:F];  s_dst = X @ a[F:]
    e[i, j] = leaky_relu(s_src[i] + s_dst[j], alpha=0.2)

Sharding over 8 NeuronCores: row-shard the N x N output (1024 rows/core).
No collective: each core receives a replicated fp16 copy of X^T and
computes the full s_dst row-vector itself with full-rate fp16 matmuls
whose ones-like stationary operand broadcasts s_dst across all 128
partitions directly (d_bcast).  The local s_src slice comes from an
f32 row shard via fused DVE multiply-reduce.

Per-core dataflow (half = 4096 output columns):
  - a_cols [128,4] f32 via a strided DMA of the attention vector; the
    stationary lhsT tiles are ones*a_dst chunks (tensor_scalar).
  - per half: 8x2 fp16 matmuls (K=256 in two 128-passes) -> PSUM
    [128,512] holding s_dst replicated across partitions -> DVE copy
    casts to an fp16 d_bcast [128,4096] tile.
  - main loop per half: 8 sub-rows t; row r=t*128+p of the local block.
      ACT rows: out = Prelu(d_bcast + s_src[:,t]) in one activation.
      DVE rows: u = d+s; v = 0.2*d+0.2*s; out = max(u,v).
    Output tile [128,4096] fp16 -> DMA (8 KiB/partition lines, 1 MiB).

Output is written in fp16 (harness gate is 2e-2 rel err; fp16 keeps it
~1e-4) and upcast to f32 on the host, halving the dominant HBM write
traffic.  Roofline: ~22 MB DMA/core at ~358 GB/s -> ~61 us floor.
"""

import numpy as np

N = 8192
F = 256
NCORES = 8
ROWS = N // NCORES          # 1024 rows per core
P = 128                     # partitions
C = ROWS // P               # 8 sub-rows per partition
ALPHA = 0.2
HALF = N // 2               # 4096 columns per half
QCH = 512                   # matmul free-dim chunk (one PSUM bank)
NACT = 5                    # sub-rows on the scalar engine (rest on DVE)

_CACHE = {}


def _build():
    import concourse.bacc as bacc
    import concourse.bass as bass
    import concourse.tile as tile
    from concourse import mybir

    fp32 = mybir.dt.float32
    fp16 = mybir.dt.float16

    nc = bacc.Bacc(
        "TRN2",
        target_bir_lowering=False,
        debug=False,
        num_devices=NCORES,
    )

    xt_dram = nc.dram_tensor("xt", [F, N], fp16, kind="ExternalInput")
    xloc_dram = nc.dram_tensor("xloc", [P, C * F], fp32, kind="ExternalInput")
    av_dram = nc.dram_tensor("av", [2 * F, 1], fp32, kind="ExternalInput")
    # av_cols[f, a] = av[a*128 + f] (host-rearranged): cols 0,1 = a_src,
    # cols 2,3 = a_dst — avoids a slow 512-descriptor strided DMA
    avc_dram = nc.dram_tensor("av_cols", [P, 4], fp32, kind="ExternalInput")
    out_dram = nc.dram_tensor("out", [ROWS, N], fp16, kind="ExternalOutput")

    with tile.TileContext(nc) as tc:
        with (
            tc.tile_pool(name="const", bufs=1) as const_pool,
            tc.tile_pool(name="xt", bufs=2) as xt_pool,
            tc.tile_pool(name="work", bufs=2) as work_pool,
            tc.tile_pool(name="dbc", bufs=2) as dbc_pool,
            tc.tile_pool(name="uv", bufs=4) as uv_pool,
            tc.tile_pool(name="outp", bufs=6) as out_pool,
            tc.tile_pool(name="psum", bufs=4, space=bass.MemorySpace.PSUM) as psum_pool,
            tc.tile_pool(name="psA", bufs=2, space=bass.MemorySpace.PSUM) as psA_pool,
        ):
            # ---- tiny loads on the scalar ring; they gate lhsT / s_src ----
            a_cols = const_pool.tile([P, 4], fp32)
            nc.scalar.dma_start(a_cols[:], avc_dram.ap())
            # flat row copy of av for the a_src partition-broadcast matmul
            av_row = const_pool.tile([1, 2 * F], fp32)
            nc.scalar.dma_start(av_row[:], av_dram.ap().rearrange("f one -> one f"))

            # ---- xloc first on the sync ring (small; gates s_src mults),
            # then the X^T halves that feed d_bcast ----
            xloc_sb = const_pool.tile([P, C * F], fp32)
            nc.sync.dma_start(xloc_sb[:], xloc_dram.ap())

            xtA = [xt_pool.tile([P, HALF], fp16, tag="xtA", name=f"xtA{h}") for h in range(2)]
            xtB = [xt_pool.tile([P, HALF], fp16, tag="xtB", name=f"xtB{h}") for h in range(2)]
            xt_ap = xt_dram.ap()
            for h in range(2):
                nc.sync.dma_start(xtA[h][:], xt_ap[0:P, h * HALF:(h + 1) * HALF])
                nc.sync.dma_start(xtB[h][:], xt_ap[P:2 * P, h * HALF:(h + 1) * HALF])

            # ---- stationary lhsT tiles: ones * a_dst chunk ----
            ones16 = const_pool.tile([P, P], fp16)
            nc.vector.memset(ones16[:], 1.0)
            lhsT = []
            for kb in range(2):
                t_ = const_pool.tile([P, P], fp16, tag=f"lhsT{kb}", name=f"lhsT{kb}")
                nc.vector.tensor_scalar(
                    t_[:], ones16[:], a_cols[:, 2 + kb:3 + kb], None,
                    op0=mybir.AluOpType.mult,
                )
                lhsT.append(t_)

            # ---- a_src broadcast across partitions (for the s_src matvec) ----
            ones1 = const_pool.tile([1, P], fp32)
            nc.vector.memset(ones1[:], 1.0)
            asrc_ps = psum_pool.tile([P, F], fp32, tag="asrc")
            nc.tensor.matmul(
                asrc_ps[:], ones1[:], av_row[0:1, 0:F], start=True, stop=True
            )
            ab_src = const_pool.tile([P, F], fp32)
            nc.vector.tensor_copy(ab_src[:], asrc_ps[:])

            # ---- s_src: GpSimd multiplies (free-dim reduce must run on the
            # vector engine; those are emitted after the first d_bcast
            # copies so they don't stall the DVE FIFO) ----
            # (tensor_tensor_reduce faults real HW; keep the two-op form)
            s_src = const_pool.tile([P, C], fp32)
            mv_scratch = const_pool.tile([P, C * F], fp32)
            for c in range(C):
                nc.gpsimd.tensor_tensor(
                    mv_scratch[:, c * F:(c + 1) * F],
                    xloc_sb[:, c * F:(c + 1) * F], ab_src[:],
                    op=mybir.AluOpType.mult,
                )

            def emit_ssrc_reduces():
                for c in range(C):
                    nc.vector.tensor_reduce(
                        s_src[:, c:c + 1], mv_scratch[:, c * F:(c + 1) * F],
                        axis=mybir.AxisListType.X, op=mybir.AluOpType.add,
                    )

            s_srcA = const_pool.tile([P, C], fp32)

            out_view = out_dram.ap().rearrange("(c p) n -> p c n", c=C)

            for h in range(2):
                # ---- d_bcast[h]: s_dst replicated across partitions, fp16 ----
                dbc = dbc_pool.tile([P, HALF], fp16, tag="dbc")
                for q in range(HALF // QCH):
                    ps = psA_pool.tile([P, QCH], fp32, tag="dps")
                    sl = slice(q * QCH, (q + 1) * QCH)
                    nc.tensor.matmul(
                        ps[:], lhsT[0][:], xtA[h][:, sl], start=True, stop=False
                    )
                    nc.tensor.matmul(
                        ps[:], lhsT[1][:], xtB[h][:, sl], start=False, stop=True
                    )
                    nc.vector.tensor_copy(dbc[:, sl], ps[:])

                if h == 0:
                    emit_ssrc_reduces()
                    nc.gpsimd.tensor_scalar(
                        s_srcA[:], s_src[:], ALPHA, None,
                        op0=mybir.AluOpType.mult,
                    )

                # ---- main loop: 8 sub-rows over this half ----
                csl = slice(h * HALF, (h + 1) * HALF)
                for t in range(C):
                    o = out_pool.tile([P, HALF], fp16)
                    if t < NACT:
                        nc.scalar.activation(
                            o[:],
                            dbc[:],
                            mybir.ActivationFunctionType.Prelu,
                            bias=s_src[:, t:t + 1],
                            scale=1.0,
                            alpha=ALPHA,
                        )
                    else:
                        u = uv_pool.tile([P, HALF], fp16, tag="u")
                        v = uv_pool.tile([P, HALF], fp16, tag="v")
                        nc.vector.tensor_scalar(
                            u[:], dbc[:], s_src[:, t:t + 1], None,
                            op0=mybir.AluOpType.add,
                        )
                        nc.vector.tensor_scalar(
                            v[:], dbc[:], ALPHA, s_srcA[:, t:t + 1],
                            op0=mybir.AluOpType.mult, op1=mybir.AluOpType.add,
                        )
                        nc.vector.tensor_tensor(
                            o[:], u[:], v[:], op=mybir.AluOpType.max
                        )
                    eng = nc.sync if t % 2 == 0 else nc.scalar
                    eng.dma_start(out_view[:, t, csl], o[:])

    nc.compile()
    return nc


def _get_nc():
    if "nc" not in _CACHE:
        _CACHE["nc"] = _build()
    return _CACHE["nc"]


def build_in_maps(feature_matrix: np.ndarray, attention_vector: np.ndarray):
    feature_matrix = np.ascontiguousarray(feature_matrix, dtype=np.float32)
    attention_vector = np.ascontiguousarray(attention_vector, dtype=np.float32)
    xt = np.ascontiguousarray(feature_matrix.T.astype(np.float16))
    av_cols = np.ascontiguousarray(attention_vector.reshape(4, P).T)
    in_maps = []
    for c in range(NCORES):
        shard = feature_matrix[c * ROWS:(c + 1) * ROWS]
        # partition p, sub-row chunk c2 holds local row c2*128 + p
        xloc = np.ascontiguousarray(
            shard.reshape(C, P, F).transpose(1, 0, 2).reshape(P, C * F)
        )
        in_maps.append({
            "xt": xt,
            "xloc": xloc,
            "av": attention_vector,
            "av_cols": av_cols,
        })
    return in_maps


def kernel(feature_matrix: np.ndarray, attention_vector: np.ndarray) -> np.ndarray:
    from concourse.bass_utils import run_bass_kernel_spmd

    nc = _get_nc()
    in_maps = build_in_maps(feature_matrix, attention_vector)
    res = run_bass_kernel_spmd(nc, in_maps, core_ids=list(range(NCORES)))
    out = np.concatenate(
        [res.results[c]["out"] for c in range(NCORES)], axis=0
    )
    return out.astype(np.float32)
